# revision 27
# baseline (speedup 1.0000x reference)
"""Bass/Trainium2 kernel for BatchedDiffPoolLayer (8 NeuronCores).

Hardcoded for N=12288, E=196608, F=128, ASSIGN=256, 8 graphs, 8 cores.

Strategy:
  - Nodes sharded across 8 cores (1536/core). Edges routed to the core
    owning the segment-sum target (dst for the conv aggregation, src for
    the A@S aggregation) and sorted, so each 128-edge chunk maps into one
    node group; segment-sum = dma_gather (HBM row gather) + one-hot
    matmul accumulation on TensorE (f32r).
  - The two SAGE convs share the aggregation; their dense layers fuse
    into one [256 -> 384] matmul.
  - The NxN link-pred residual is never materialized:
      ||A - S S^T||_F^2 = sum(A^2) - 2*sum_i S_i.(A S)_i + ||S^T S||_F^2
    with sum(A^2) counted via shifted-equality on the sorted edge list.
  - Cross-core: AllGather of the result shard, one AllReduce of
    [adj_new | h | G | scalars].
  - entropy_loss depends only on the mask block width (CPG=32) and N:
    computed as the same f32 constant the reference produces.
"""

import numpy as np

N = 12288
E = 196608
F = 128          # in/out feats
A = 256          # assign dim
NGRAPH = 8
CPG = A // NGRAPH
NC = 8
NLOC = N // NC   # 1536
G1 = 256         # phase-1 node-group width (6 groups/core)
G2 = 128         # phase-2 node-group width (12 groups/core)
NG1 = NLOC // G1
NG2 = NLOC // G2
DUP_W = 8

_prog_cache = {}


def _round_up(v, m):
    return (v + m - 1) // m * m


def _wrap16(arr, eg):
    w = arr.astype(np.int16).reshape(eg // 16, 16).T.copy()
    return np.tile(w, (8, 1))


def _wrap128(arr, eg):
    return arr.astype(np.float32).reshape(eg // 128, 128).T.copy()


def _build_program(EG1, EG2, debug=False, po=15):
    import concourse.bass as bass
    import concourse.bacc as bacc
    import concourse.mybir as mybir
    import concourse.tile as tile
    from concourse.masks import make_identity

    f32 = mybir.dt.float32
    f32r = mybir.dt.float32r
    i16 = mybir.dt.int16
    i32 = mybir.dt.int32
    Alu = mybir.AluOpType
    Act = mybir.ActivationFunctionType
    Ax = mybir.AxisListType

    W1C = EG1 // 128
    CB2 = EG2 // 128          # chunks per (group,pair) bucket
    GW2 = 4 * EG2             # padded edges per phase-2 group
    L2 = NG2 * GW2
    C2 = L2 // 128

    nc = bacc.Bacc(None, num_devices=NC)

    x_d = nc.dram_tensor("x", [N, F], f32r, kind="ExternalInput")
    xloc_d = nc.dram_tensor("xloc", [NLOC, F], f32, kind="ExternalInput")
    wcat_d = nc.dram_tensor("wcat", [2 * F, F + A], f32, kind="ExternalInput")
    bcat_d = nc.dram_tensor("bcat", [1, F + A], f32, kind="ExternalInput")
    gidx1_d = nc.dram_tensor("gidx1", [NG1, 128, EG1 // 16], i16, kind="ExternalInput")
    drel1_d = nc.dram_tensor("drel1", [NG1, 128, W1C], f32, kind="ExternalInput")
    gidx2_d = nc.dram_tensor("gidx2", [NG2, 128, GW2 // 16], i16, kind="ExternalInput")
    srel2_d = nc.dram_tensor("srel2", [NG2, 128, 4 * CB2], f32, kind="ExternalInput")
    batchf_d = nc.dram_tensor("batchf", [128, NG2], f32, kind="ExternalInput")
    batchp_d = nc.dram_tensor("batchp", [128, NG2], f32, kind="ExternalInput")
    dups_d = nc.dram_tensor("dups", [L2 + 16], f32, kind="ExternalInput")
    dupd_d = nc.dram_tensor("dupd", [L2 + 16], f32, kind="ExternalInput")

    adj_out = nc.dram_tensor("adj_new", [A, A], f32, kind="ExternalOutput")
    h_out_d = nc.dram_tensor("h_out", [A, F], f32, kind="ExternalOutput")
    loss_out = nc.dram_tensor("adj_loss", [1, 1], f32, kind="ExternalOutput")

    if debug:
        aggTo_d = nc.dram_tensor("aggTo", [128, NLOC], f32, kind="ExternalOutput")
        xTo_d = nc.dram_tensor("xTo", [128, NLOC], f32, kind="ExternalOutput")
        ho_d = nc.dram_tensor("ho", [NLOC, F + A], f32, kind="ExternalOutput")
        embo_d = nc.dram_tensor("embo", [NLOC, F], f32, kind="ExternalOutput")
        reso_d = nc.dram_tensor("reso", [NLOC, A], f32, kind="ExternalOutput")
        aro_d = nc.dram_tensor("aro", [NLOC, A], f32, kind="ExternalOutput")

    res_shard = nc.dram_tensor("res_shard", [NLOC, A], f32r)
    resc_shard = nc.dram_tensor("resc_shard", [NLOC, 64], f32r)
    resc_full = nc.dram_tensor("resc_full", [N, 64], f32r, addr_space="Shared")
    red_in = nc.dram_tensor("red_in", [128, 904], f32)
    red_out = nc.dram_tensor("red_out", [128, 904], f32, addr_space="Shared")

    RG = [list(range(NC))]

    with tile.TileContext(nc) as tc:
        with tc.tile_pool(name="const", bufs=1) as cpool:
            ident = cpool.tile([128, 128], f32)
            make_identity(nc, ident[:])
            iota256i = cpool.tile([128, 256], i32)
            nc.gpsimd.iota(iota256i[:], pattern=[[1, 256]], base=0, channel_multiplier=0)
            iota256f = cpool.tile([128, 256], f32)
            nc.vector.tensor_copy(iota256f[:], iota256i[:])
            iota128f = cpool.tile([128, 128], f32)
            nc.vector.tensor_copy(iota128f[:], iota256i[:, :128])
            colgi = cpool.tile([128, 256], i32)
            nc.gpsimd.iota(colgi[:], pattern=[[1, 8], [0, 32]], base=0, channel_multiplier=0)
            colg = cpool.tile([128, 256], f32)
            nc.vector.tensor_copy(colg[:], colgi[:])
            ones_colf = cpool.tile([128, 1], f32)
            nc.vector.memset(ones_colf[:], 1.0)
            ones_col = cpool.tile([128, 1], f32r)
            nc.vector.tensor_copy(ones_col[:], ones_colf[:])
            w1_sb = cpool.tile([128, F + A], f32)
            nc.sync.dma_start(out=w1_sb[:], in_=wcat_d[:128, :])
            w2_sb = cpool.tile([128, F + A], f32)
            nc.sync.dma_start(out=w2_sb[:], in_=wcat_d[128:, :])
            w1r = cpool.tile([128, F + A], f32r)
            nc.vector.tensor_copy(w1r[:], w1_sb[:])
            w2r = cpool.tile([128, F + A], f32r)
            nc.vector.tensor_copy(w2r[:], w2_sb[:])
            ones_row = cpool.tile([1, 128], f32)
            nc.vector.memset(ones_row[:], 1.0)
            brow = cpool.tile([1, F + A], f32)
            nc.sync.dma_start(out=brow[:], in_=bcat_d[:, :])
            brow2 = cpool.tile([1, F + A], f32)
            nc.vector.tensor_copy(brow2[:], brow[:])
            bias_t = cpool.tile([128, F + A], f32)
            batch_t = cpool.tile([128, NG2], f32)
            nc.sync.dma_start(out=batch_t[:], in_=batchf_d[:, :])
            batchp_t = cpool.tile([128, NG2], f32)
            nc.sync.dma_start(out=batchp_t[:], in_=batchp_d[:, :])

            xT_all = cpool.tile([128, NLOC], f32r)
            aggT_all = cpool.tile([128, NLOC], f32r)
            deg_all = cpool.tile([1, NLOC], f32)
            rcp_all = cpool.tile([128, NG2], f32)
            res_all = cpool.tile([128, NG2 * A], f32r)
            rd_acc = cpool.tile([128, 1], f32)
            nc.vector.memset(rd_acc[:], 0.0)
            red_sb = cpool.tile([128, 904], f32)
            nc.vector.memset(red_sb[:], 0.0)

            with tc.tile_pool(name="bias_ps", bufs=1, space="PSUM") as bps:
                bias_psum = bps.tile([128, F + A], f32)
                nc.tensor.matmul(out=bias_psum[:], lhsT=ones_row[:], rhs=brow2[:],
                                 start=True, stop=True)
                nc.scalar.copy(bias_t[:], bias_psum[:])

            # ---------- Phase 1: agg & deg (gather + one-hot matmul) ----------
            with (
                tc.tile_pool(name="p1", bufs=3) as p1,
                tc.tile_pool(name="p1b", bufs=4) as p1b,
                tc.tile_pool(name="p1ps", bufs=2, space="PSUM") as p1ps,
                tc.tile_pool(name="p1psd", bufs=2, space="PSUM") as p1psd,
                tc.tile_pool(name="p1rc", bufs=2, space="PSUM") as p1rc,
            ):
                for g in range(NG1 if po & 1 else 0):
                    idx_t = p1.tile([128, EG1 // 16], i16, tag="idx")
                    nc.sync.dma_start(out=idx_t[:], in_=gidx1_d[g, :, :])
                    drel_t = p1.tile([128, W1C], f32, tag="drel")
                    nc.sync.dma_start(out=drel_t[:], in_=drel1_d[g, :, :])
                    v_t = p1.tile([128, EG1], f32r, tag="v")
                    for s in range(0, EG1, 896):
                        n = min(896, EG1 - s)
                        nc.gpsimd.dma_gather(
                            out_ap=v_t[:, (s // 128) * F:((s + n) // 128) * F]
                                .rearrange("p (g f) -> p g f", f=F),
                            in_ap=x_d[:, :],
                            idxs_ap=idx_t[:, s // 16:(s + n) // 16],
                            num_idxs=n,
                            num_idxs_reg=n,
                            elem_size=F,
                        )
                    aggT_ps = p1ps.tile([128, G1], f32, tag="aggT")
                    deg_ps = p1psd.tile([1, G1], f32, tag="deg")
                    for k in range(W1C):
                        b_t = p1b.tile([128, G1], f32r, tag="b1")
                        nc.vector.tensor_scalar(
                            b_t[:], iota256f[:], drel_t[:, k:k + 1], None,
                            op0=Alu.is_equal,
                        )
                        nc.tensor.matmul(
                            out=aggT_ps[:],
                            lhsT=v_t[:, k * F:(k + 1) * F],
                            rhs=b_t[:],
                            start=(k == 0), stop=(k == W1C - 1),
                        )
                        nc.tensor.matmul(
                            out=deg_ps[:],
                            lhsT=ones_col[:],
                            rhs=b_t[:],
                            start=(k == 0), stop=(k == W1C - 1),
                        )
                    nc.vector.tensor_scalar(deg_all[:, g * G1:(g + 1) * G1],
                                            deg_ps[:], 1.0, None, op0=Alu.max)
                    nc.vector.tensor_copy(aggT_all[:, g * G1:(g + 1) * G1], aggT_ps[:])

                # xT via PE transpose of local x rows
                for j in range(NG2 if po & 2 else 0):
                    xr = p1.tile([128, F], f32, tag="xr")
                    nc.sync.dma_start(out=xr[:], in_=xloc_d[j * 128:(j + 1) * 128, :])
                    tps = p1psd.tile([128, 128], f32, tag="tps")
                    nc.tensor.transpose(out=tps[:], in_=xr[:], identity=ident[:])
                    nc.vector.tensor_copy(xT_all[:, j * 128:(j + 1) * 128], tps[:])

                for j in range(NG2 if po & 2 else 0):
                    rcp_ps = p1rc.tile([128, 1], f32, tag="rcps")
                    nc.tensor.transpose(out=rcp_ps[:],
                                        in_=deg_all[:1, j * 128:(j + 1) * 128],
                                        identity=ident[:1, :1])
                    nc.vector.reciprocal(rcp_all[:, j:j + 1], rcp_ps[:])

            if debug:
                nc.sync.dma_start(out=aggTo_d[:, :].bitcast(f32r), in_=aggT_all[:])
                nc.sync.dma_start(out=xTo_d[:, :].bitcast(f32r), in_=xT_all[:])

            # ---------- Phase 3: dense + normalize + softmax + h/G ----------
            with (
                tc.tile_pool(name="p3", bufs=3) as p3,
                tc.tile_pool(name="p3ps", bufs=2, space="PSUM") as p3ps,
                tc.tile_pool(name="p3acc", bufs=1, space="PSUM") as p3acc,
            ):
                h0_ps = p3acc.tile([128, F], f32)
                h1_ps = p3acc.tile([128, F], f32)
                g0_ps = p3acc.tile([128, 64], f32)
                g1_ps = p3acc.tile([128, 64], f32)
                for j in range(NG2 if po & 2 else 0):
                    hps = p3ps.tile([128, F + A], f32, tag="dense")
                    nc.tensor.matmul(
                        out=hps[:], lhsT=ones_row[:], rhs=brow2[:],
                        start=True, stop=False,
                    )
                    nc.tensor.matmul(
                        out=hps[:],
                        lhsT=xT_all[:, j * 128:(j + 1) * 128],
                        rhs=w1r[:], start=False, stop=True,
                    )
                    hps2 = p3ps.tile([128, F + A], f32, tag="dense2")
                    nc.tensor.matmul(
                        out=hps2[:],
                        lhsT=aggT_all[:, j * 128:(j + 1) * 128],
                        rhs=w2r[:], start=True, stop=True,
                    )
                    hx_sb = p3.tile([128, F + A], f32, tag="hx")
                    nc.scalar.copy(hx_sb[:], hps[:])
                    h_sb = p3.tile([128, F + A], f32, tag="h")
                    nc.vector.scalar_tensor_tensor(
                        out=h_sb[:], in0=hps2[:], scalar=rcp_all[:, j:j + 1],
                        in1=hx_sb[:], op0=Alu.mult, op1=Alu.add,
                    )

                    sq = p3.tile([128, F], f32, tag="sq")
                    ss = p3.tile([128, 1], f32, tag="ss")
                    nc.scalar.activation(sq[:], h_sb[:, :F], Act.Square, accum_out=ss[:])
                    nrm = p3.tile([128, 1], f32, tag="nrm")
                    nc.scalar.activation(nrm[:], ss[:], Act.Sqrt)
                    den = p3.tile([128, 1], f32, tag="den")
                    nc.vector.tensor_scalar(den[:], nrm[:], 1e-12, None, op0=Alu.max)
                    rcp = p3.tile([128, 1], f32, tag="rcp")
                    nc.vector.reciprocal(rcp[:], den[:])
                    embed_sb = p3.tile([128, F], f32r, tag="embed")
                    nc.vector.tensor_scalar(embed_sb[:], h_sb[:, :F], rcp[:], None, op0=Alu.mult)

                    maskt = p3.tile([128, A], f32, tag="mask")
                    nc.vector.tensor_scalar(maskt[:], colg[:], batch_t[:, j:j + 1], None, op0=Alu.is_equal)
                    z_t = p3.tile([128, A], f32, tag="z")
                    nc.vector.tensor_tensor(z_t[:], h_sb[:, F:], maskt[:], op=Alu.mult)
                    negm = p3.tile([128, 1], f32, tag="negm")
                    nc.vector.tensor_reduce(negm[:], z_t[:], axis=Ax.X, op=Alu.max, negate=True)
                    e_t = p3.tile([128, A], f32, tag="e")
                    s_t = p3.tile([128, 1], f32, tag="s")
                    nc.scalar.activation(e_t[:], z_t[:], Act.Exp, bias=negm[:], accum_out=s_t[:])
                    em_t = p3.tile([128, A], f32, tag="em")
                    sm_t = p3.tile([128, 1], f32, tag="sm")
                    nc.vector.scalar_tensor_tensor(
                        out=em_t[:], in0=e_t[:], scalar=1.0, in1=maskt[:],
                        op0=Alu.bypass, op1=Alu.mult, accum_out=sm_t[:],
                    )
                    den2 = p3.tile([128, 1], f32, tag="den2")
                    nc.vector.scalar_tensor_tensor(
                        out=den2[:], in0=s_t[:], scalar=1e-13, in1=sm_t[:],
                        op0=Alu.mult, op1=Alu.add,
                    )
                    rcp2 = p3.tile([128, 1], f32, tag="rcp2")
                    nc.vector.reciprocal(rcp2[:], den2[:])
                    res_slab = res_all[:, j * A:(j + 1) * A]
                    nc.vector.tensor_scalar(res_slab, em_t[:], rcp2[:], None, op0=Alu.mult)
                    nc.sync.dma_start(out=res_shard[j * 128:(j + 1) * 128, :], in_=res_slab)
                    if debug:
                        nc.sync.dma_start(out=ho_d[j * 128:(j + 1) * 128, :], in_=h_sb[:])
                        nc.sync.dma_start(out=embo_d[j * 128:(j + 1) * 128, :].bitcast(f32r), in_=embed_sb[:])
                        nc.sync.dma_start(out=reso_d[j * 128:(j + 1) * 128, :].bitcast(f32r), in_=res_slab)

                    nc.tensor.matmul(out=h0_ps[:], lhsT=res_all[:, j * A:j * A + 128],
                                     rhs=embed_sb[:], start=(j == 0), stop=(j == NG2 - 1))
                    nc.tensor.matmul(out=h1_ps[:], lhsT=res_all[:, j * A + 128:(j + 1) * A],
                                     rhs=embed_sb[:], start=(j == 0), stop=(j == NG2 - 1))
                    resc_t = p3.tile([128, 64], f32r, tag="resc")
                    for p in range(4):
                        pm = p3.tile([128, 1], f32, tag=f"pm{p}")
                        nc.vector.tensor_scalar(pm[:], batchp_t[:, j:j + 1], float(p),
                                                None, op0=Alu.is_equal)
                        if p == 0:
                            nc.vector.tensor_scalar(resc_t[:], res_all[:, j * A:j * A + 64],
                                                    pm[:], None, op0=Alu.mult)
                        else:
                            nc.vector.scalar_tensor_tensor(
                                out=resc_t[:], in0=res_all[:, j * A + 64 * p:j * A + 64 * p + 64],
                                scalar=pm[:], in1=resc_t[:], op0=Alu.mult, op1=Alu.add)
                    nc.sync.dma_start(out=resc_shard[j * 128:(j + 1) * 128, :], in_=resc_t[:])
                    nc.tensor.matmul(out=g0_ps[:], lhsT=res_all[:, j * A:j * A + 128],
                                     rhs=resc_t[:], start=(j == 0), stop=(j == NG2 - 1))
                    nc.tensor.matmul(out=g1_ps[:], lhsT=res_all[:, j * A + 128:(j + 1) * A],
                                     rhs=resc_t[:], start=(j == 0), stop=(j == NG2 - 1))

                if po & 2:
                    nc.vector.tensor_copy(red_sb[:, 512:640], h0_ps[:])
                    nc.vector.tensor_copy(red_sb[:, 640:768], h1_ps[:])
                    nc.vector.tensor_copy(red_sb[:, 768:832], g0_ps[:])
                    nc.vector.tensor_copy(red_sb[:, 832:896], g1_ps[:])

            if po & 8:
                nc.gpsimd.collective_compute(
                    "AllGather", Alu.bypass, replica_groups=RG,
                    ins=[resc_shard[:, :]], outs=[resc_full[:, :]],
                )

            # ---------- Phase 2: Ar = A@S rows, adj_new, rowdot ----------
            with (
                tc.tile_pool(name="p2", bufs=3) as p2,
                tc.tile_pool(name="p2b", bufs=4) as p2b,
                tc.tile_pool(name="p2ps", bufs=2, space="PSUM") as p2ps,
                tc.tile_pool(name="p2acc", bufs=1, space="PSUM") as p2acc,
            ):
                adj0_ps = p2acc.tile([128, A], f32)
                adj1_ps = p2acc.tile([128, A], f32)
                for j in range(NG2 if po & 4 else 0):
                    idx_t = p2.tile([128, GW2 // 16], i16, tag="idx2")
                    nc.sync.dma_start(out=idx_t[:], in_=gidx2_d[j, :, :])
                    srel_t = p2.tile([128, 4 * CB2], f32, tag="srel")
                    nc.sync.dma_start(out=srel_t[:], in_=srel2_d[j, :, :])
                    v_t = p2.tile([128, (GW2 // 128) * 64], f32r, tag="vr")
                    for s in range(0, GW2, 896):
                        n = min(896, GW2 - s)
                        nc.gpsimd.dma_gather(
                            out_ap=v_t[:, (s // 128) * 64:((s + n) // 128) * 64]
                                .rearrange("p (g f) -> p g f", f=64),
                            in_ap=resc_full[:, :],
                            idxs_ap=idx_t[:, s // 16:(s + n) // 16],
                            num_idxs=n,
                            num_idxs_reg=n,
                            elem_size=64,
                        )
                    ar_ps = p2ps.tile([128, A], f32, tag="ar")
                    for k in range(4 * CB2):
                        p = k // CB2
                        b_t = p2b.tile([128, 128], f32r, tag="b2")
                        nc.vector.tensor_scalar(
                            b_t[:], iota128f[:], srel_t[:, k:k + 1], None,
                            op0=Alu.is_equal,
                        )
                        nc.tensor.matmul(
                            out=ar_ps[:, 64 * p:64 * (p + 1)],
                            lhsT=b_t[:],
                            rhs=v_t[:, k * 64:(k + 1) * 64],
                            start=(k % CB2 == 0), stop=(k % CB2 == CB2 - 1),
                        )
                    ar_sb = p2.tile([128, A], f32r, tag="arsb")
                    nc.vector.tensor_copy(ar_sb[:], ar_ps[:])
                    if debug:
                        nc.sync.dma_start(out=aro_d[j * 128:(j + 1) * 128, :].bitcast(f32r), in_=ar_sb[:])
                    scr = p2.tile([128, A], f32, tag="scr")
                    rdj = p2.tile([128, 1], f32, tag="rdj")
                    nc.vector.scalar_tensor_tensor(
                        out=scr[:], in0=res_all[:, j * A:(j + 1) * A], scalar=1.0,
                        in1=ar_sb[:], op0=Alu.bypass, op1=Alu.mult, accum_out=rdj[:],
                    )
                    nc.vector.tensor_tensor(rd_acc[:], rd_acc[:], rdj[:], op=Alu.add)
                    nc.tensor.matmul(out=adj0_ps[:], lhsT=res_all[:, j * A:j * A + 128],
                                     rhs=ar_sb[:], start=(j == 0), stop=(j == NG2 - 1))
                    nc.tensor.matmul(out=adj1_ps[:], lhsT=res_all[:, j * A + 128:(j + 1) * A],
                                     rhs=ar_sb[:], start=(j == 0), stop=(j == NG2 - 1))

                # duplicate-pair count via shifted equality
                if not (po & 4):
                    pass
                s0 = p2.tile([128, C2], f32, tag="s0")
                nc.sync.dma_start(out=s0[:], in_=dups_d[0:L2].rearrange("(p f) -> p f", p=128))
                d0 = p2.tile([128, C2], f32, tag="d0")
                nc.sync.dma_start(out=d0[:], in_=dupd_d[0:L2].rearrange("(p f) -> p f", p=128))
                dacc = p2.tile([128, C2], f32, tag="dacc")
                nc.vector.memset(dacc[:], 0.0)
                for k in range(1, (DUP_W if po & 4 else 0) + 1):
                    sk = p2.tile([128, C2], f32, tag="sk")
                    nc.sync.dma_start(out=sk[:], in_=dups_d[k:k + L2].rearrange("(p f) -> p f", p=128))
                    dk = p2.tile([128, C2], f32, tag="dk")
                    nc.sync.dma_start(out=dk[:], in_=dupd_d[k:k + L2].rearrange("(p f) -> p f", p=128))
                    t1 = p2.tile([128, C2], f32, tag="t1")
                    nc.vector.tensor_tensor(t1[:], sk[:], s0[:], op=Alu.is_equal)
                    t2 = p2.tile([128, C2], f32, tag="t2")
                    nc.vector.tensor_tensor(t2[:], dk[:], d0[:], op=Alu.is_equal)
                    t3 = p2.tile([128, C2], f32, tag="t3")
                    nc.vector.tensor_tensor(t3[:], t1[:], t2[:], op=Alu.mult)
                    nc.vector.tensor_tensor(dacc[:], dacc[:], t3[:], op=Alu.add)
                dupcol = p2.tile([128, 1], f32, tag="dupcol")
                if po & 4:
                    nc.vector.tensor_reduce(dupcol[:], dacc[:], axis=Ax.X, op=Alu.add)

                if po & 4:
                    nc.vector.tensor_copy(red_sb[:, 0:256], adj0_ps[:])
                    nc.vector.tensor_copy(red_sb[:, 256:512], adj1_ps[:])
                    nc.vector.tensor_copy(red_sb[:, 896:897], rd_acc[:])
                    nc.vector.tensor_copy(red_sb[:, 897:898], dupcol[:])

            nc.sync.dma_start(out=red_in[:, :], in_=red_sb[:])
            if po & 8:
                nc.gpsimd.collective_compute(
                    "AllReduce", Alu.add, replica_groups=RG,
                    ins=[red_in[:, :]], outs=[red_out[:, :]],
                )

            # ---------- Finalize ----------
            with (
                tc.tile_pool(name="fin", bufs=1) as fin,
                tc.tile_pool(name="finps", bufs=1, space="PSUM") as finps,
            ):
                red2 = fin.tile([128, 904], f32)
                nc.sync.dma_start(out=red2[:], in_=red_out[:, :])
                scrg = fin.tile([128, 128], f32)
                g2c = fin.tile([128, 1], f32)
                nc.scalar.activation(scrg[:], red2[:, 768:896], Act.Square, accum_out=g2c[:])
                vec3 = fin.tile([128, 3], f32)
                nc.scalar.copy(vec3[:, 0:1], red2[:, 896:897])
                nc.scalar.copy(vec3[:, 1:2], red2[:, 897:898])
                nc.scalar.copy(vec3[:, 2:3], g2c[:])
                srow = fin.tile([1, 3], f32)
                nc.gpsimd.tensor_reduce(srow[:], vec3[:], axis=Ax.C, op=Alu.add)
                t1s = fin.tile([1, 1], f32)
                nc.vector.tensor_scalar(t1s[:], srow[:1, 1:2], 2.0, float(E), op0=Alu.mult, op1=Alu.add)
                t2s = fin.tile([1, 1], f32)
                nc.vector.scalar_tensor_tensor(
                    out=t2s[:], in0=srow[:1, 0:1], scalar=-2.0, in1=t1s[:],
                    op0=Alu.mult, op1=Alu.add,
                )
                t3s = fin.tile([1, 1], f32)
                nc.vector.tensor_tensor(t3s[:], t2s[:], srow[:1, 2:3], op=Alu.add)
                t4s = fin.tile([1, 1], f32)
                nc.vector.tensor_scalar(t4s[:], t3s[:], 0.0, None, op0=Alu.max)
                t5s = fin.tile([1, 1], f32)
                nc.scalar.activation(t5s[:], t4s[:], Act.Sqrt)
                t6s = fin.tile([1, 1], f32)
                nc.vector.tensor_scalar(t6s[:], t5s[:], 1.0 / float(N) ** 2, None, op0=Alu.mult)
                nc.sync.dma_start(out=loss_out[:, :], in_=t6s[:])

                nc.sync.dma_start(out=adj_out[:128, :], in_=red2[:, 0:256])
                nc.sync.dma_start(out=adj_out[128:, :], in_=red2[:, 256:512])
                nc.sync.dma_start(out=h_out_d[:128, :], in_=red2[:, 512:640])
                nc.sync.dma_start(out=h_out_d[128:, :], in_=red2[:, 640:768])

    nc.finalize()
    return nc


def _prepare_inputs(x, edge_index, batch, W_embd, b_embd, W_pool, b_pool):
    src = np.asarray(edge_index[0], dtype=np.int64)
    dst = np.asarray(edge_index[1], dtype=np.int64)
    x = np.ascontiguousarray(np.asarray(x, dtype=np.float32))

    # phase 1: sorted by (dst, src), grouped by 256-node windows of dst
    o1 = np.lexsort((src, dst))
    d1, s1 = dst[o1], src[o1]
    b1 = np.searchsorted(d1, np.arange(0, N + 1, G1))
    EG1 = max(128, _round_up(int(np.diff(b1).max()), 128))

    # phase 2: sorted by (src-group, dst-pair, src, dst); buckets of
    # (128-src-node group, dst block-pair), each padded to EG2
    batch_np = np.asarray(batch, dtype=np.int64)
    pairarr = (batch_np // 2)[dst]
    srcgrp = src // G2
    o2 = np.lexsort((dst, src, pairarr, srcgrp))
    s2, d2 = src[o2], dst[o2]
    key2 = (srcgrp * 4 + pairarr)[o2]
    b2 = np.searchsorted(key2, np.arange(0, (N // G2) * 4 + 1))
    EG2 = max(128, _round_up(int(np.diff(b2).max()), 128))

    GW2 = 4 * EG2
    L2 = NG2 * GW2
    wcat = np.ascontiguousarray(
        np.concatenate([np.asarray(W_embd, np.float32), np.asarray(W_pool, np.float32)], axis=1))
    bcat = np.ascontiguousarray(
        np.concatenate([np.asarray(b_embd, np.float32), np.asarray(b_pool, np.float32)])[None, :])

    in_maps = []
    for c in range(NC):
        gidx1 = np.zeros((NG1, 128, EG1 // 16), np.int16)
        drel1 = np.zeros((NG1, 128, EG1 // 128), np.float32)
        for g in range(NG1):
            gi = c * NG1 + g
            lo, hi = b1[gi], b1[gi + 1]
            n = hi - lo
            idx = np.zeros(EG1, np.int64)
            rel = np.full(EG1, 999.0, np.float64)
            idx[:n] = s1[lo:hi]
            rel[:n] = d1[lo:hi] - (c * NLOC + g * G1)
            gidx1[g] = _wrap16(idx, EG1)
            drel1[g] = _wrap128(rel, EG1)

        gidx2 = np.zeros((NG2, 128, GW2 // 16), np.int16)
        srel2 = np.zeros((NG2, 128, GW2 // 128), np.float32)
        dups = np.full(L2 + 16, -1.0, np.float32)
        dupd = -(np.arange(L2 + 16, dtype=np.float32) + 5.0)
        for g in range(NG2):
            idx = np.zeros(GW2, np.int64)
            rel = np.full(GW2, 999.0, np.float64)
            for p in range(4):
                bi = (c * NG2 + g) * 4 + p
                lo, hi = b2[bi], b2[bi + 1]
                n = hi - lo
                off = p * EG2
                idx[off:off + n] = d2[lo:hi]
                rel[off:off + n] = s2[lo:hi] - (c * NLOC + g * G2)
                dups[g * GW2 + off:g * GW2 + off + n] = s2[lo:hi].astype(np.float32)
                dupd[g * GW2 + off:g * GW2 + off + n] = d2[lo:hi].astype(np.float32)
            gidx2[g] = _wrap16(idx, GW2)
            srel2[g] = _wrap128(rel, GW2)

        batchf = np.ascontiguousarray(
            np.asarray(batch[c * NLOC:(c + 1) * NLOC], np.float32).reshape(NG2, 128).T)
        batchp = np.ascontiguousarray(
            (batch_np[c * NLOC:(c + 1) * NLOC] // 2).astype(np.float32).reshape(NG2, 128).T)

        in_maps.append(dict(
            x=x,
            xloc=np.ascontiguousarray(x[c * NLOC:(c + 1) * NLOC]),
            wcat=wcat, bcat=bcat,
            gidx1=gidx1, drel1=drel1,
            gidx2=gidx2, srel2=srel2,
            batchf=batchf, batchp=batchp, dups=dups, dupd=dupd,
        ))
    return in_maps, EG1, EG2


def _entropy_const():
    p = np.float32(1.0) / np.float32(CPG)
    t = p * np.log(p)
    row = np.float32(-CPG) * t
    return np.float32(N) * row


def run(x, edge_index, batch, W_embd, b_embd, W_pool, b_pool, trace=False, debug=False):
    from concourse.bass_utils import run_bass_kernel_spmd

    in_maps, EG1, EG2 = _prepare_inputs(x, edge_index, batch, W_embd, b_embd, W_pool, b_pool)
    key = (EG1, EG2, debug)
    if key not in _prog_cache:
        _prog_cache[key] = _build_program(EG1, EG2, debug=debug)
    nc = _prog_cache[key]
    res = run_bass_kernel_spmd(nc, in_maps, core_ids=list(range(NC)), trace=trace)
    r0 = res.results[0]
    adj_new = r0["adj_new"].astype(np.float32)
    h = r0["h_out"].astype(np.float32)
    adj_loss = np.float32(r0["adj_loss"][0, 0])
    entropy = _entropy_const()
    return (adj_new, h, adj_loss, entropy), res


def kernel(x, edge_index, batch, W_embd, b_embd, W_pool, b_pool):
    out, _ = run(x, edge_index, batch, W_embd, b_embd, W_pool, b_pool, trace=False)
    return out


def estimate_time_ns():
    """Cost-model (TimelineSim) estimate for the most recently built program."""
    try:
        from concourse.timeline_sim import TimelineSim
    except Exception:
        return None
    if not _prog_cache:
        return None
    nc = next(iter(_prog_cache.values()))
    try:
        return int(TimelineSim(nc, trace=False).simulate())
    except Exception:
        return None
